# revision 29
# baseline (speedup 1.0000x reference)
"""Contrastive-learning loss kernel for 8 TRN2 NeuronCores.

loss = logsumexp(sim_neg / T) - mean(sim_pos) / T

where sim_pos/sim_neg are all-pairs cosine similarities. Two reductions:
  denom = sum_ij exp(s_i . r_j / T)        (needs the full N x N matmul)
  sum(sim_pos) = (sum_i s_i) . (sum_j b_j) (rank-1 identity, no matmul)
with s/r/b = row-normalized x_source / x_raw_target / x_bc_target.

Sharding (2 x 4 grid over the sim_neg matrix): core c = a*4+b gets
S rows [a*2048, (a+1)*2048) and R rows [b*1024, (b+1)*1024), plus a
distinct 512-row shard of x_bc_target for the numerator partial (the
x_source numerator shard is rows b*512.. of its own S block, which the
host rotates to the front). Each core returns partial exp-sums and
inv-norm-weighted row-sums; the host combines the 8 partials in
float64 and takes the log.

Fast path (tolerance is 2e-2 relative; fp8 keeps us ~1e-4):
 - s/r/b ship as fp8e4 (host cast). S and b additionally ship
   PRE-TRANSPOSED (host layout transform, like the row rotation), so
   the kernel never transposes them; sim matmuls run fp8 DoubleRow
   (2 contraction k-tiles per instruction, 0.5 cyc/row).
 - S/b row norms come from PE Gram diagonals: G = xT^T @ xT per
   128-row chunk (DoubleRow, accumulated over k in PSUM), then ONE
   fused DVE pass (tensor_tensor_reduce against the identity) extracts
   and row-sums the diagonal. ~10us total instead of ~40us of
   elementwise square+reduce passes.
 - R is transposed on device, and its normalization is applied BY the
   transpose: each [128,128] transpose is a regular matmul
   x_chunk^T @ diag(C/||row||) with a per-tile fp8 diagonal. R ssq
   uses fused square+reduce passes split DVE/ACT.
 - rsqrt is a linear seed + 2 Newton steps on DVE, so ACT never needs
   a non-exp table function (square/copy/exp share the exp_and_others
   act set -> one table load total). S norms ride in the per-partition
   ACT exp scale.
"""

import json

import numpy as np
import ml_dtypes

import concourse.bass as bass
import concourse.mybir as mybir
import concourse.tile as tile
from concourse.bass_utils import run_bass_kernel_spmd
from concourse.masks import make_identity
from concourse.vector_clock import ScopedClock, VectorClock

P = 128
N = 4096
D = 2048
TEMP = 0.5
A_SPLIT = 2  # S-row blocks
B_SPLIT = 4  # R-row blocks
SB = N // A_SPLIT  # 2048 source rows per core
RB = N // B_SPLIT  # 1024 raw-target rows per core
NSH = N // 8  # 512 numerator-shard rows per core
KT = D // P  # 16 contraction tiles
ST = SB // P  # 16 source row-tiles per core
RT = RB // P  # 8 raw row-tiles per core
JB = RB // 512  # 2 psum-bank columns of the sim block
CSC = 64.0  # fp8 scale: normalized rows are stored as C * x / ||x||

F32 = mybir.dt.float32
FP8 = mybir.dt.float8e4
AF = mybir.ActivationFunctionType


def _spill_sync_waits(raw: bytes) -> bytes:
    """The walrus here has no sync-wait slots on Matmult (fused weight-load
    S3_LW struct) and chokes on multi-wait instructions generally. Move every
    Matmult wait — and all but the first wait of any other instruction — onto
    single-wait NoOps inserted just before it on the same engine queue."""
    d = json.loads(raw)
    ctr = 0
    for fn in d["functions"]:
        for blk in fn["blocks"]:
            out = []
            for inst in blk["instructions"]:
                si = inst.get("sync_info")
                waits = si.get("on_wait") if si else None
                limit = 0 if inst.get("opcode") == "Matmult" else 1
                if waits and len(waits) > limit:
                    for w in waits[limit:]:
                        ctr += 1
                        out.append(
                            {
                                "debug": inst.get("debug"),
                                "engine": inst["engine"],
                                "ins": [],
                                "name": f"I-waitfix-{ctr}",
                                "opcode": "NoOp",
                                "outs": [],
                                "sync_info": {"on_update": [], "on_wait": [w]},
                            }
                        )
                    si["on_wait"] = waits[:limit]
                out.append(inst)
            blk["instructions"] = out
    return json.dumps(d).encode()


class PatchedBass(bass.Bass):
    def to_json_bytes(self) -> bytes:
        return _spill_sync_waits(super().to_json_bytes())


class TC(tile.TileContext):
    """TileContext whose kernel-tail drain carries its sem waits on
    single-wait NOPs — this walrus rejects multi-wait Drain instructions."""

    def _drain_and_barrier(self, tick_clock, wait_clock):
        g = tick_clock.global_clock
        nprocs = len(g)
        for p in range(nprocs):
            t = g[p]
            if t <= 0:
                continue
            vec = [0] * nprocs
            vec[p] = t
            nop = self.nc.sync.nop(nofuse=True)
            wait_clock.add_sem_waits(nop.ins, ScopedClock({None: VectorClock(vec)}))
        self.nc.sync.drain()
        self.nc.all_engine_barrier()
        assert self.sems is not None
        popped = self.nc._tile_sem_poison_stack.pop()
        assert popped is self._sem_poison
        self.nc.clear_and_free_semaphores(list(self.sems.allocated().values()))
        self.nc.all_engine_barrier()


def build(double_row=True):
    from concourse.alu_op_type import AluOpType as OP

    nc = PatchedBass()
    sT_block = nc.dram_tensor("sT_block", [D, SB], FP8, kind="ExternalInput")
    s_shard = nc.dram_tensor("s_shard", [NSH, D], FP8, kind="ExternalInput")
    r_block = nc.dram_tensor("r_block", [RB, D], FP8, kind="ExternalInput")
    b_shard = nc.dram_tensor("b_shard", [NSH, D], FP8, kind="ExternalInput")
    bT_shard = nc.dram_tensor("bT_shard", [D, NSH], FP8, kind="ExternalInput")
    denom_acc = nc.dram_tensor("denom_acc", [P, ST * JB], F32, kind="ExternalOutput")
    ssum = nc.dram_tensor("ssum", [1, D], F32, kind="ExternalOutput")
    bsum = nc.dram_tensor("bsum", [1, D], F32, kind="ExternalOutput")

    DRM = mybir.MatmulPerfMode.DoubleRow if double_row else None

    with TC(nc) as tc:
        with (
            tc.tile_pool(name="big", bufs=1) as big,
            tc.tile_pool(name="rxp", bufs=RT) as rxp,
            tc.tile_pool(name="sqp", bufs=2) as sqp,
            tc.tile_pool(name="escp", bufs=2) as escp,
            tc.tile_pool(name="vecp", bufs=3) as vecp,
            tc.tile_pool(name="tpsum", bufs=2, space="PSUM") as tpsum,
            tc.tile_pool(name="gpsum", bufs=3, space="PSUM") as gpsum,
            tc.tile_pool(name="grsum", bufs=2, space="PSUM") as grsum,
            tc.tile_pool(name="vpsum", bufs=1, space="PSUM") as vpsum,
        ):
            identF = big.tile([P, P], F32, name="identF")
            make_identity(nc, identF)
            ident8 = big.tile([P, P], FP8, name="ident8")
            with nc.allow_low_precision(reason="exact 1.0 in fp8"):
                nc.vector.tensor_copy(out=ident8, in_=identF)
            rTn = big.tile([P, KT, RB], FP8, name="rTn")
            sTall = big.tile([P, KT, SB], FP8, name="sTall")
            bTall = big.tile([P, KT, NSH], FP8, name="bTall")
            s4 = big.tile([P, 4, D], FP8, name="s4")
            b4 = big.tile([P, 4, D], FP8, name="b4")
            dacc = big.tile([P, ST * JB], F32, name="dacc")

            def ssq_dve(x, ssqg, col):
                """ssqg[:, col] = sum_f x*x (times 1/D) via DVE bn_stats
                (mean/var in one pass; ssq/D = var + mean^2)."""
                nch = D // 512
                stats = vecp.tile([P, nch, 6], F32, tag="stats", name="stats")
                xr = x.rearrange("p (c f) -> p c f", c=nch)
                for c4 in range(nch):
                    nc.vector.bn_stats(out=stats[:, c4, :], in_=xr[:, c4, :])
                mv = vecp.tile([P, 2], F32, tag="mv", name="mv")
                nc.vector.bn_aggr(out=mv, in_=stats)
                m2 = vecp.tile([P, 1], F32, tag="m2", name="m2")
                nc.vector.tensor_mul(m2, mv[:, 0:1], mv[:, 0:1])
                nc.vector.tensor_add(ssqg[:, col : col + 1], mv[:, 1:2], m2)

            def ssq_act(x, ssqg, col):
                """ssq/D via ACT Square+accum (scale folds 1/sqrt(D) inside
                the square); square is in every act set."""
                sq = sqp.tile([P, D], mybir.dt.bfloat16, tag="sqa", name="sqa")
                with nc.allow_low_precision(reason="squares scratch"):
                    nc.scalar.activation(
                        out=sq,
                        in_=x,
                        func=AF.Square,
                        scale=float(1.0 / np.sqrt(D)),
                        accum_out=ssqg[:, col : col + 1],
                    )
                return sq

            def gram_chunk(xT, c, ssqg, col, label):
                """ssqg[:, col] = ssq of rows [c*128,(c+1)*128) from the
                TRANSPOSED layout: PE Gram (DoubleRow over k) + one DVE
                diag-extract pass (mult by identity, row-sum accum)."""
                G = grsum.tile([P, P], F32, tag="gram", name=f"G_{label}{c}")
                if double_row:
                    for q in range(KT // 2):
                        nc.tensor.matmul(
                            G,
                            lhsT=xT[:, 2 * q : 2 * q + 2, c * P : (c + 1) * P],
                            rhs=xT[:, 2 * q : 2 * q + 2, c * P : (c + 1) * P],
                            start=q == 0,
                            stop=q == KT // 2 - 1,
                            perf_mode=DRM,
                        )
                else:
                    for k in range(KT):
                        nc.tensor.matmul(
                            G,
                            lhsT=xT[:, k, c * P : (c + 1) * P],
                            rhs=xT[:, k, c * P : (c + 1) * P],
                            start=k == 0,
                            stop=k == KT - 1,
                        )
                junk = sqp.tile([P, P], F32, tag="gjunk", name="gjunk")
                nc.vector.tensor_mul(junk, G, identF)
                nc.vector.reduce_sum(
                    ssqg[:, col : col + 1], junk, axis=mybir.AxisListType.X
                )

            def newton_rsqrt(z, n, prescale, lo, hi, label):
                """y = rsqrt(prescale * z[:, :n]) via linear seed + 2 Newton
                steps (DVE only, no ACT tables). prescale*z must be in
                [lo, hi]."""
                flo, fhi = 1.0 / np.sqrt(lo), 1.0 / np.sqrt(hi)
                slope = (fhi - flo) / (hi - lo)
                icpt = flo - slope * lo
                y = vecp.tile([P, n], F32, tag=f"nw_y{label}", name=f"y_{label}")
                u = vecp.tile([P, n], F32, tag=f"nw_u{label}", name=f"u_{label}")
                nc.vector.tensor_scalar(
                    out=y, in0=z[:, :n], scalar1=slope * prescale, scalar2=icpt,
                    op0=OP.mult, op1=OP.add,
                )
                for _ in range(2):
                    nc.vector.tensor_mul(u, y, y)
                    nc.vector.tensor_mul(u, u, z[:, :n])
                    nc.vector.tensor_scalar(
                        out=u, in0=u, scalar1=-0.5 * prescale, scalar2=1.5,
                        op0=OP.mult, op1=OP.add,
                    )
                    nc.vector.tensor_mul(y, y, u)
                return y

            def numerator_dr(xt4, iv8, out_dram, label):
                """out_dram[1, D] = sum_i iv8[i] * xt4[i, :] over 512 rows
                (4 stacked tiles), DoubleRow pairs, chunk-major (2 live
                psum tiles)."""
                for cc in range(4):
                    vch = vpsum.tile([1, 512], F32, tag="vch", name=f"v_{label}{cc}")
                    for t in range(4):
                        nc.tensor.matmul(
                            vch,
                            lhsT=iv8[:, t : t + 1],
                            rhs=xt4[:, t, cc * 512 : (cc + 1) * 512],
                            start=t == 0,
                            stop=t == 3,
                        )
                    osb = vecp.tile([1, 512], F32, tag="osb", name=f"osb_{label}{cc}")
                    nc.vector.tensor_copy(out=osb, in_=vch)
                    nc.sync.dma_start(
                        out=out_dram[:, cc * 512 : (cc + 1) * 512], in_=osb
                    )

            # ---- input DMAs: R first (gates the sim matmuls), then sT in
            # 4 column groups (one [128, 16, 512] transfer each, so the
            # first Gram chunks can start after ~1MB instead of 4MB), then
            # the numerator shards as single whole-tensor transfers. Few,
            # large transfers: each dma_start costs ~0.6us of SP issue time.
            rxs = []
            for jt in range(RT):
                rx = rxp.tile([P, D], FP8, tag="rx", name="rx")
                nc.sync.dma_start(out=rx, in_=r_block[jt * P : (jt + 1) * P, :])
                rxs.append(rx)
            # Serialize the DMA waves so R gets full bandwidth first: a tiny
            # gpsimd op reading the last R tile gates the sT transfers
            # (issued from the gpsimd queue), and one reading sT gates the
            # numerator shards.
            guard1 = big.tile([P, 1], FP8, name="guard1")
            with nc.allow_low_precision(reason="dma ordering guard"):
                nc.gpsimd.tensor_copy(out=guard1, in_=rxs[RT - 1][:, 0:1])
            sT_kp = sT_block.rearrange("(k p) f -> p k f", p=P)
            for g in range(4):
                nc.gpsimd.dma_start(
                    out=sTall[:, :, g * 512 : (g + 1) * 512],
                    in_=sT_kp[:, :, g * 512 : (g + 1) * 512],
                )
            guard2 = big.tile([P, 1], FP8, name="guard2")
            with nc.allow_low_precision(reason="dma ordering guard"):
                nc.gpsimd.tensor_copy(out=guard2, in_=sTall[:, KT - 1, D - 1 : D])
            nc.gpsimd.dma_start(
                out=s4, in_=s_shard.rearrange("(t p) f -> p t f", p=P)
            )
            nc.gpsimd.dma_start(
                out=bTall, in_=bT_shard.rearrange("(k p) f -> p k f", p=P)
            )
            nc.gpsimd.dma_start(
                out=b4, in_=b_shard.rearrange("(t p) f -> p t f", p=P)
            )

            # ---- R: ssq passes (DVE/ACT split) and Newton in batches of 2
            # tiles so the first diag(C/||r||) scale-transposes start ~6us
            # in; transposes write rTn.
            for jp in range(RT // 2):
                ssq_r = vecp.tile([P, 2], F32, tag="ssq_r", name=f"ssq_r{jp}")
                for h in range(2):
                    (ssq_dve if h == 0 else ssq_act)(rxs[2 * jp + h], ssq_r, h)
                # ssq_r holds ssq/D; prescale D/C^2 -> ssq/C^2 in [0.38,0.64]
                inv_r = newton_rsqrt(
                    ssq_r, 2, D / (CSC * CSC), 0.38, 0.64, f"r{jp}"
                )
                for h in range(2):
                    jt = 2 * jp + h
                    rx = rxs[jt]
                    dg = vecp.tile([P, P], FP8, tag="diag", name=f"diag_r{jt}")
                    with nc.allow_low_precision(reason="fp8 matmul operand"):
                        nc.vector.tensor_scalar_mul(dg, ident8, inv_r[:, h : h + 1])
                    for kb in range(KT // 4):
                        tp = tpsum.tile([P, 512], F32, tag="tp", name="tp")
                        for q in range(4):
                            k = kb * 4 + q
                            nc.tensor.matmul(
                                tp[:, q * P : (q + 1) * P],
                                lhsT=rx[:, k * P : (k + 1) * P],
                                rhs=dg,
                                start=True,
                                stop=True,
                            )
                        dst = rTn[:, kb * 4 : (kb + 1) * 4, jt * P : (jt + 1) * P]
                        src = tp.rearrange("p (a b) -> p a b", a=4)
                        with nc.allow_low_precision(reason="fp8 matmul operand"):
                            if kb % 2 == 0:
                                nc.vector.tensor_copy(out=dst, in_=src)
                            else:
                                nc.scalar.copy(out=dst, in_=src)

            # ---- S norms via Gram diagonals (all 16, then one Newton batch
            # — interleaving Newton between sim groups measured WORSE: the
            # DVE-side chain stalls exps and bubbles the PE).
            ssq_s = vecp.tile([P, ST], F32, tag="ssq_s", name="ssq_s")
            for c in range(ST):
                gram_chunk(sTall, c, ssq_s, c, "s")
            y_s = newton_rsqrt(ssq_s, ST, 1.0 / (CSC * CSC), 0.38, 0.64, "s")
            # psum = C*(s_i . r_hat_j) and y_s = C/||s||, so the exp scale
            # y_s/(C^2*T) makes the argument cos/T.
            sc_s = vecp.tile([P, ST], F32, tag="sc_s", name="sc_s")
            nc.vector.tensor_scalar(
                out=sc_s, in0=y_s, scalar1=1.0 / (CSC * CSC * TEMP), scalar2=0.0,
                op0=OP.mult, op1=OP.add,
            )
            # numerator needs fp8(C/||s||) for the first 4 chunks (the
            # host-rotated own-shard rows)
            iv8s = vecp.tile([P, 4], FP8, tag="iv8s", name="iv8s")
            with nc.allow_low_precision(reason="fp8 matmul operand"):
                nc.vector.tensor_copy(out=iv8s, in_=y_s[:, 0:4])

            # ---- sim matmuls + exp
            col_of = lambda st, jb: st * JB + jb
            for st in range(ST):
                gs = [
                    gpsum.tile([P, 512], F32, tag="g", name="g") for _ in range(JB)
                ]
                for jb in range(JB):
                    # jb-outer: the first tiles' jb=0 matmuls only need R
                    # tiles 0-3 transposed, so sims start earlier.
                    if double_row:
                        for q in range(KT // 2):
                            nc.tensor.matmul(
                                gs[jb],
                                lhsT=sTall[:, 2 * q : 2 * q + 2, st * P : (st + 1) * P],
                                rhs=rTn[:, 2 * q : 2 * q + 2, jb * 512 : (jb + 1) * 512],
                                start=q == 0,
                                stop=q == KT // 2 - 1,
                                perf_mode=DRM,
                            )
                    else:
                        for k in range(KT):
                            nc.tensor.matmul(
                                gs[jb],
                                lhsT=sTall[:, k, st * P : (st + 1) * P],
                                rhs=rTn[:, k, jb * 512 : (jb + 1) * 512],
                                start=k == 0,
                                stop=k == KT - 1,
                            )
                    esc = escp.tile([P, 512], F32, tag="esc", name="esc")
                    nc.scalar.activation(
                        out=esc,
                        in_=gs[jb],
                        func=AF.Exp,
                        scale=sc_s[:, st : st + 1],
                        accum_out=dacc[:, col_of(st, jb) : col_of(st, jb) + 1],
                    )

            # ---- numerators (off the critical path, after the sims)
            numerator_dr(s4, iv8s, ssum, "s")
            ssq_b = vecp.tile([P, 4], F32, tag="ssq_b", name="ssq_b")
            for c4 in range(4):
                gram_chunk(bTall, c4, ssq_b, c4, "b")
            # prescale*ssq = ssq/C^2 in [0.38, 0.64]
            y_b = newton_rsqrt(ssq_b, 4, 1.0 / (CSC * CSC), 0.38, 0.64, "b")
            iv8b = vecp.tile([P, 4], FP8, tag="iv8b", name="iv8b")
            with nc.allow_low_precision(reason="fp8 matmul operand"):
                nc.vector.tensor_copy(out=iv8b, in_=y_b)
            numerator_dr(b4, iv8b, bsum, "b")

            nc.sync.dma_start(out=denom_acc[:, :], in_=dacc)
    return nc


_NC_CACHE = {}


def _get_nc():
    if "nc" not in _NC_CACHE:
        _NC_CACHE["nc"] = build()
    return _NC_CACHE["nc"]


def _make_in_maps(x_source, x_bc_target, x_raw_target):
    fp8 = ml_dtypes.float8_e4m3
    s8 = np.asarray(x_source, dtype=np.float32).astype(fp8)
    r8 = np.asarray(x_raw_target, dtype=np.float32).astype(fp8)
    b8 = np.asarray(x_bc_target, dtype=np.float32).astype(fp8)
    in_maps = []
    for c in range(8):
        a, b = c // B_SPLIT, c % B_SPLIT
        sblk = s8[a * SB : (a + 1) * SB]
        # Rotate so the core's numerator shard (local rows b*512..(b+1)*512)
        # lands in tiles [0, 4) — the kernel always numerates its first 4.
        sblk = np.concatenate(
            [sblk[b * NSH : (b + 1) * NSH], sblk[: b * NSH], sblk[(b + 1) * NSH :]],
            axis=0,
        )
        bs = b8[c * NSH : (c + 1) * NSH]
        in_maps.append(
            {
                "sT_block": np.ascontiguousarray(sblk.T),
                "s_shard": np.ascontiguousarray(sblk[:NSH]),
                "r_block": np.ascontiguousarray(r8[b * RB : (b + 1) * RB]),
                "b_shard": np.ascontiguousarray(bs),
                "bT_shard": np.ascontiguousarray(bs.T),
            }
        )
    return in_maps


def _combine(results):
    denom = 0.0
    s_tot = np.zeros(D, dtype=np.float64)
    b_tot = np.zeros(D, dtype=np.float64)
    for r in results:
        denom += r["denom_acc"].astype(np.float64).sum()
        s_tot += r["ssum"][0].astype(np.float64)
        b_tot += r["bsum"][0].astype(np.float64)
    s_tot /= CSC  # numerator lhsT was fp8(C/||s||)
    b_tot /= CSC  # numerator lhsT was fp8(C/||b||)
    loss = np.log(denom) - (s_tot @ b_tot) / (float(N) * float(N)) / TEMP
    return np.array(loss, dtype=np.float32)


def _run(x_source, x_bc_target, x_raw_target, trace=False):
    nc = _get_nc()
    in_maps = _make_in_maps(x_source, x_bc_target, x_raw_target)
    res = run_bass_kernel_spmd(nc, in_maps, core_ids=list(range(8)), trace=trace)
    return _combine(res.results), res


def kernel(x_source, x_bc_target, x_raw_target):
    out, _ = _run(x_source, x_bc_target, x_raw_target)
    return out


# revision 33
# speedup vs baseline: 1.0334x; 1.0334x over previous
"""Contrastive-learning loss kernel for 8 TRN2 NeuronCores.

loss = logsumexp(sim_neg / T) - mean(sim_pos) / T

where sim_pos/sim_neg are all-pairs cosine similarities. Two reductions:
  denom = sum_ij exp(s_i . r_j / T)        (needs the full N x N matmul)
  sum(sim_pos) = (sum_i s_i) . (sum_j b_j) (rank-1 identity, no matmul)
with s/r/b = row-normalized x_source / x_raw_target / x_bc_target.

Sharding (2 x 4 grid over the sim_neg matrix): core c = a*4+b gets
S rows [a*2048, (a+1)*2048) and R rows [b*1024, (b+1)*1024), plus a
distinct 512-row shard of x_bc_target for the numerator partial (the
x_source numerator shard is rows b*512.. of its own S block, which the
host rotates to the front). Each core returns partial exp-sums and
inv-norm-weighted row-sums; the host combines the 8 partials in
float64 and takes the log.

Fast path (tolerance is 2e-2 relative; fp8 keeps us ~1e-4):
 - s/r/b ship as fp8e4 (host cast). S and b additionally ship
   PRE-TRANSPOSED (host layout transform, like the row rotation), so
   the kernel never transposes them; sim matmuls run fp8 DoubleRow
   (2 contraction k-tiles per instruction, 0.5 cyc/row).
 - S/b row norms come from PE Gram diagonals: G = xT^T @ xT per
   128-row chunk (DoubleRow, accumulated over k in PSUM), then ONE
   fused DVE pass (tensor_tensor_reduce against the identity) extracts
   and row-sums the diagonal. ~10us total instead of ~40us of
   elementwise square+reduce passes.
 - R is transposed on device, and its normalization is applied BY the
   transpose: each [128,128] transpose is a regular matmul
   x_chunk^T @ diag(C/||row||) with a per-tile fp8 diagonal. R ssq
   uses fused square+reduce passes split DVE/ACT.
 - rsqrt is a linear seed + 2 Newton steps on DVE, so ACT never needs
   a non-exp table function (square/copy/exp share the exp_and_others
   act set -> one table load total). S norms ride in the per-partition
   ACT exp scale.
"""

import json

import numpy as np
import ml_dtypes

import concourse.bass as bass
import concourse.mybir as mybir
import concourse.tile as tile
from concourse.bass_utils import run_bass_kernel_spmd
from concourse.masks import make_identity
from concourse.vector_clock import ScopedClock, VectorClock

P = 128
N = 4096
D = 2048
TEMP = 0.5
A_SPLIT = 2  # S-row blocks
B_SPLIT = 4  # R-row blocks
SB = N // A_SPLIT  # 2048 source rows per core
RB = N // B_SPLIT  # 1024 raw-target rows per core
NSH = N // 8  # 512 numerator-shard rows per core
KT = D // P  # 16 contraction tiles
ST = SB // P  # 16 source row-tiles per core
RT = RB // P  # 8 raw row-tiles per core
JB = RB // 512  # 2 psum-bank columns of the sim block
CSC = 64.0  # fp8 scale: normalized rows are stored as C * x / ||x||

F32 = mybir.dt.float32
FP8 = mybir.dt.float8e4
AF = mybir.ActivationFunctionType


def _spill_sync_waits(raw: bytes) -> bytes:
    """The walrus here has no sync-wait slots on Matmult (fused weight-load
    S3_LW struct) and chokes on multi-wait instructions generally. Move every
    Matmult wait — and all but the first wait of any other instruction — onto
    single-wait NoOps inserted just before it on the same engine queue."""
    d = json.loads(raw)
    ctr = 0
    for fn in d["functions"]:
        for blk in fn["blocks"]:
            out = []
            for inst in blk["instructions"]:
                si = inst.get("sync_info")
                waits = si.get("on_wait") if si else None
                limit = 0 if inst.get("opcode") == "Matmult" else 1
                if waits and len(waits) > limit:
                    for w in waits[limit:]:
                        ctr += 1
                        out.append(
                            {
                                "debug": inst.get("debug"),
                                "engine": inst["engine"],
                                "ins": [],
                                "name": f"I-waitfix-{ctr}",
                                "opcode": "NoOp",
                                "outs": [],
                                "sync_info": {"on_update": [], "on_wait": [w]},
                            }
                        )
                    si["on_wait"] = waits[:limit]
                out.append(inst)
            blk["instructions"] = out
    return json.dumps(d).encode()


class PatchedBass(bass.Bass):
    def to_json_bytes(self) -> bytes:
        return _spill_sync_waits(super().to_json_bytes())


class TC(tile.TileContext):
    """TileContext whose kernel-tail drain carries its sem waits on
    single-wait NOPs — this walrus rejects multi-wait Drain instructions."""

    def _drain_and_barrier(self, tick_clock, wait_clock):
        g = tick_clock.global_clock
        nprocs = len(g)
        for p in range(nprocs):
            t = g[p]
            if t <= 0:
                continue
            vec = [0] * nprocs
            vec[p] = t
            nop = self.nc.sync.nop(nofuse=True)
            wait_clock.add_sem_waits(nop.ins, ScopedClock({None: VectorClock(vec)}))
        self.nc.sync.drain()
        self.nc.all_engine_barrier()
        assert self.sems is not None
        popped = self.nc._tile_sem_poison_stack.pop()
        assert popped is self._sem_poison
        self.nc.clear_and_free_semaphores(list(self.sems.allocated().values()))
        self.nc.all_engine_barrier()


def build(double_row=True):
    from concourse.alu_op_type import AluOpType as OP

    nc = PatchedBass()
    sT_block = nc.dram_tensor("sT_block", [D, SB], FP8, kind="ExternalInput")
    s_shard = nc.dram_tensor("s_shard", [NSH, D], FP8, kind="ExternalInput")
    r_block = nc.dram_tensor("r_block", [RB, D], FP8, kind="ExternalInput")
    b_shard = nc.dram_tensor("b_shard", [NSH, D], FP8, kind="ExternalInput")
    bT_shard = nc.dram_tensor("bT_shard", [D, NSH], FP8, kind="ExternalInput")
    denom_acc = nc.dram_tensor("denom_acc", [P, ST * JB], F32, kind="ExternalOutput")
    ssum = nc.dram_tensor("ssum", [1, D], F32, kind="ExternalOutput")
    bsum = nc.dram_tensor("bsum", [1, D], F32, kind="ExternalOutput")

    DRM = mybir.MatmulPerfMode.DoubleRow if double_row else None

    with TC(nc) as tc:
        with (
            tc.tile_pool(name="big", bufs=1) as big,
            tc.tile_pool(name="rxp", bufs=RT) as rxp,
            tc.tile_pool(name="sqp", bufs=2) as sqp,
            tc.tile_pool(name="escp", bufs=2) as escp,
            tc.tile_pool(name="vecp", bufs=3) as vecp,
            tc.tile_pool(name="tpsum", bufs=2, space="PSUM") as tpsum,
            tc.tile_pool(name="gpsum", bufs=2, space="PSUM") as gpsum,
            tc.tile_pool(name="grsum", bufs=2, space="PSUM") as grsum,
            tc.tile_pool(name="vpsum", bufs=2, space="PSUM") as vpsum,
        ):
            identF = big.tile([P, P], F32, name="identF")
            make_identity(nc, identF)
            ident8 = big.tile([P, P], FP8, name="ident8")
            with nc.allow_low_precision(reason="exact 1.0 in fp8"):
                nc.vector.tensor_copy(out=ident8, in_=identF)
            rTn = big.tile([P, KT, RB], FP8, name="rTn")
            sTall = big.tile([P, KT, SB], FP8, name="sTall")
            bTall = big.tile([P, KT, NSH], FP8, name="bTall")
            s4 = big.tile([P, 4, D], FP8, name="s4")
            b4 = big.tile([P, 4, D], FP8, name="b4")
            dacc = big.tile([P, ST * JB], F32, name="dacc")

            def ssq_dve(x, ssqg, col):
                """ssqg[:, col] = sum_f x*x (times 1/D) via DVE bn_stats
                (mean/var in one pass; ssq/D = var + mean^2)."""
                nch = D // 512
                stats = vecp.tile([P, nch, 6], F32, tag="stats", name="stats")
                xr = x.rearrange("p (c f) -> p c f", c=nch)
                for c4 in range(nch):
                    nc.vector.bn_stats(out=stats[:, c4, :], in_=xr[:, c4, :])
                mv = vecp.tile([P, 2], F32, tag="mv", name="mv")
                nc.vector.bn_aggr(out=mv, in_=stats)
                m2 = vecp.tile([P, 1], F32, tag="m2", name="m2")
                nc.vector.tensor_mul(m2, mv[:, 0:1], mv[:, 0:1])
                nc.vector.tensor_add(ssqg[:, col : col + 1], mv[:, 1:2], m2)

            def ssq_act(x, ssqg, col):
                """ssq/D via ACT Square+accum (scale folds 1/sqrt(D) inside
                the square); square is in every act set."""
                sq = sqp.tile([P, D], mybir.dt.bfloat16, tag="sqa", name="sqa")
                with nc.allow_low_precision(reason="squares scratch"):
                    nc.scalar.activation(
                        out=sq,
                        in_=x,
                        func=AF.Square,
                        scale=float(1.0 / np.sqrt(D)),
                        accum_out=ssqg[:, col : col + 1],
                    )
                return sq

            def gram_chunk(xT, c, ssqg, col, label):
                """ssqg[:, col] = ssq of rows [c*128,(c+1)*128) from the
                TRANSPOSED layout: PE Gram (DoubleRow over k) + one DVE
                diag-extract pass (mult by identity, row-sum accum)."""
                G = grsum.tile([P, P], F32, tag="gram", name=f"G_{label}{c}")
                if double_row:
                    for q in range(KT // 2):
                        nc.tensor.matmul(
                            G,
                            lhsT=xT[:, 2 * q : 2 * q + 2, c * P : (c + 1) * P],
                            rhs=xT[:, 2 * q : 2 * q + 2, c * P : (c + 1) * P],
                            start=q == 0,
                            stop=q == KT // 2 - 1,
                            perf_mode=DRM,
                        )
                else:
                    for k in range(KT):
                        nc.tensor.matmul(
                            G,
                            lhsT=xT[:, k, c * P : (c + 1) * P],
                            rhs=xT[:, k, c * P : (c + 1) * P],
                            start=k == 0,
                            stop=k == KT - 1,
                        )
                junk = sqp.tile([P, P], F32, tag="gjunk", name="gjunk")
                nc.vector.tensor_mul(junk, G, identF)
                nc.vector.reduce_sum(
                    ssqg[:, col : col + 1], junk, axis=mybir.AxisListType.X
                )

            def newton_rsqrt(z, n, prescale, lo, hi, label):
                """y = rsqrt(prescale * z[:, :n]) via linear seed + 2 Newton
                steps (DVE only, no ACT tables). prescale*z must be in
                [lo, hi]."""
                flo, fhi = 1.0 / np.sqrt(lo), 1.0 / np.sqrt(hi)
                slope = (fhi - flo) / (hi - lo)
                icpt = flo - slope * lo
                y = vecp.tile([P, n], F32, tag=f"nw_y{label}", name=f"y_{label}")
                u = vecp.tile([P, n], F32, tag=f"nw_u{label}", name=f"u_{label}")
                nc.vector.tensor_scalar(
                    out=y, in0=z[:, :n], scalar1=slope * prescale, scalar2=icpt,
                    op0=OP.mult, op1=OP.add,
                )
                for _ in range(2):
                    nc.vector.tensor_mul(u, y, y)
                    nc.vector.tensor_mul(u, u, z[:, :n])
                    nc.vector.tensor_scalar(
                        out=u, in0=u, scalar1=-0.5 * prescale, scalar2=1.5,
                        op0=OP.mult, op1=OP.add,
                    )
                    nc.vector.tensor_mul(y, y, u)
                return y

            def numerator_dr(xt4, iv8, out_dram, label):
                """out_dram[1, D] = sum_i iv8[i] * xt4[i, :] over 512 rows
                (4 stacked tiles), DoubleRow pairs, chunk-major (2 live
                psum tiles)."""
                for cc in range(4):
                    vch = vpsum.tile([1, 512], F32, tag="vch", name=f"v_{label}{cc}")
                    for t in range(4):
                        nc.tensor.matmul(
                            vch,
                            lhsT=iv8[:, t : t + 1],
                            rhs=xt4[:, t, cc * 512 : (cc + 1) * 512],
                            start=t == 0,
                            stop=t == 3,
                        )
                    osb = vecp.tile([1, 512], F32, tag="osb", name=f"osb_{label}{cc}")
                    nc.vector.tensor_copy(out=osb, in_=vch)
                    nc.sync.dma_start(
                        out=out_dram[:, cc * 512 : (cc + 1) * 512], in_=osb
                    )

            # ---- input DMAs: R first (gates the sim matmuls), then sT in
            # 4 column groups (one [128, 16, 512] transfer each, so the
            # first Gram chunks can start after ~1MB instead of 4MB), then
            # the numerator shards as single whole-tensor transfers. Few,
            # large transfers: each dma_start costs ~0.6us of SP issue time.
            rxs = []
            for jt in range(RT):
                rx = rxp.tile([P, D], FP8, tag="rx", name="rx")
                nc.sync.dma_start(out=rx, in_=r_block[jt * P : (jt + 1) * P, :])
                rxs.append(rx)
            # DMA hardware stripes all issued transfers across its engine
            # queues FIFO-ish by issue order — R first, then sT, then bT.
            # The s4/b4 numerator shards are issued from the SCALAR queue
            # further down, after the R ssq ops, to keep them out of the
            # critical first wave.
            sT_kp = sT_block.rearrange("(k p) f -> p k f", p=P)
            for g in range(4):
                nc.sync.dma_start(
                    out=sTall[:, :, g * 512 : (g + 1) * 512],
                    in_=sT_kp[:, :, g * 512 : (g + 1) * 512],
                )
            nc.sync.dma_start(
                out=bTall, in_=bT_shard.rearrange("(k p) f -> p k f", p=P)
            )

            # ---- R: ssq passes (DVE/ACT split) and Newton in batches of 2
            # tiles so the first diag(C/||r||) scale-transposes start ~6us
            # in; transposes write rTn.
            for jp in range(RT // 2):
                ssq_r = vecp.tile([P, 2], F32, tag="ssq_r", name=f"ssq_r{jp}")
                for h in range(2):
                    (ssq_dve if h == 0 else ssq_act)(rxs[2 * jp + h], ssq_r, h)
                # ssq_r holds ssq/D; prescale D/C^2 -> ssq/C^2 in [0.38,0.64]
                inv_r = newton_rsqrt(
                    ssq_r, 2, D / (CSC * CSC), 0.38, 0.64, f"r{jp}"
                )
                for h in range(2):
                    jt = 2 * jp + h
                    rx = rxs[jt]
                    dg = vecp.tile([P, P], FP8, tag="diag", name=f"diag_r{jt}")
                    with nc.allow_low_precision(reason="fp8 matmul operand"):
                        nc.vector.tensor_scalar_mul(dg, ident8, inv_r[:, h : h + 1])
                    for kb in range(KT // 4):
                        tp = tpsum.tile([P, 512], F32, tag="tp", name="tp")
                        for q in range(4):
                            k = kb * 4 + q
                            nc.tensor.matmul(
                                tp[:, q * P : (q + 1) * P],
                                lhsT=rx[:, k * P : (k + 1) * P],
                                rhs=dg,
                                start=True,
                                stop=True,
                            )
                        dst = rTn[:, kb * 4 : (kb + 1) * 4, jt * P : (jt + 1) * P]
                        src = tp.rearrange("p (a b) -> p a b", a=4)
                        with nc.allow_low_precision(reason="fp8 matmul operand"):
                            if kb % 2 == 0:
                                nc.vector.tensor_copy(out=dst, in_=src)
                            else:
                                nc.scalar.copy(out=dst, in_=src)

            # late shard DMAs from the scalar queue (issue ~10us in)
            nc.scalar.dma_start(
                out=s4, in_=s_shard.rearrange("(t p) f -> p t f", p=P)
            )
            nc.scalar.dma_start(
                out=b4, in_=b_shard.rearrange("(t p) f -> p t f", p=P)
            )

            # ---- S norms via Gram diagonals (all 16, then one Newton batch
            # — interleaving Newton between sim groups measured WORSE: the
            # DVE-side chain stalls exps and bubbles the PE).
            ssq_s = vecp.tile([P, ST], F32, tag="ssq_s", name="ssq_s")
            for c in range(ST):
                gram_chunk(sTall, c, ssq_s, c, "s")
            y_s = newton_rsqrt(ssq_s, ST, 1.0 / (CSC * CSC), 0.38, 0.64, "s")
            # psum = C*(s_i . r_hat_j) and y_s = C/||s||, so the exp scale
            # y_s/(C^2*T) makes the argument cos/T.
            sc_s = vecp.tile([P, ST], F32, tag="sc_s", name="sc_s")
            nc.vector.tensor_scalar(
                out=sc_s, in0=y_s, scalar1=1.0 / (CSC * CSC * TEMP), scalar2=0.0,
                op0=OP.mult, op1=OP.add,
            )
            # numerator needs fp8(C/||s||) for the first 4 chunks (the
            # host-rotated own-shard rows)
            iv8s = vecp.tile([P, 4], FP8, tag="iv8s", name="iv8s")
            with nc.allow_low_precision(reason="fp8 matmul operand"):
                nc.vector.tensor_copy(out=iv8s, in_=y_s[:, 0:4])

            # ---- sim matmuls + exp
            col_of = lambda st, jb: st * JB + jb
            for st in range(ST):
                gs = [
                    gpsum.tile([P, 512], F32, tag="g", name="g") for _ in range(JB)
                ]
                for jb in range(JB):
                    # jb-outer: the first tiles' jb=0 matmuls only need R
                    # tiles 0-3 transposed, so sims start earlier.
                    if double_row:
                        for q in range(KT // 2):
                            nc.tensor.matmul(
                                gs[jb],
                                lhsT=sTall[:, 2 * q : 2 * q + 2, st * P : (st + 1) * P],
                                rhs=rTn[:, 2 * q : 2 * q + 2, jb * 512 : (jb + 1) * 512],
                                start=q == 0,
                                stop=q == KT // 2 - 1,
                                perf_mode=DRM,
                            )
                    else:
                        for k in range(KT):
                            nc.tensor.matmul(
                                gs[jb],
                                lhsT=sTall[:, k, st * P : (st + 1) * P],
                                rhs=rTn[:, k, jb * 512 : (jb + 1) * 512],
                                start=k == 0,
                                stop=k == KT - 1,
                            )
                    esc = escp.tile([P, 512], F32, tag="esc", name="esc")
                    nc.scalar.activation(
                        out=esc,
                        in_=gs[jb],
                        func=AF.Exp,
                        scale=sc_s[:, st : st + 1],
                        accum_out=dacc[:, col_of(st, jb) : col_of(st, jb) + 1],
                    )
                if st == 7:
                    # b norms slotted mid-phase: the Gram matmuls fill PE
                    # gaps and the Newton chain runs on the idle DVE, so
                    # only the b numerator itself remains for the tail.
                    ssq_b = vecp.tile([P, 4], F32, tag="ssq_b", name="ssq_b")
                    for c4 in range(4):
                        gram_chunk(bTall, c4, ssq_b, c4, "b")
                    y_b = newton_rsqrt(
                        ssq_b, 4, 1.0 / (CSC * CSC), 0.38, 0.64, "b"
                    )
                    iv8b = vecp.tile([P, 4], FP8, tag="iv8b", name="iv8b")
                    with nc.allow_low_precision(reason="fp8 matmul operand"):
                        nc.vector.tensor_copy(out=iv8b, in_=y_b)

            # ---- numerators (off the critical path, after the sims)
            numerator_dr(s4, iv8s, ssum, "s")
            numerator_dr(b4, iv8b, bsum, "b")

            nc.sync.dma_start(out=denom_acc[:, :], in_=dacc)
    return nc


_NC_CACHE = {}


def _get_nc():
    if "nc" not in _NC_CACHE:
        _NC_CACHE["nc"] = build()
    return _NC_CACHE["nc"]


def _make_in_maps(x_source, x_bc_target, x_raw_target):
    fp8 = ml_dtypes.float8_e4m3
    s8 = np.asarray(x_source, dtype=np.float32).astype(fp8)
    r8 = np.asarray(x_raw_target, dtype=np.float32).astype(fp8)
    b8 = np.asarray(x_bc_target, dtype=np.float32).astype(fp8)
    in_maps = []
    for c in range(8):
        a, b = c // B_SPLIT, c % B_SPLIT
        sblk = s8[a * SB : (a + 1) * SB]
        # Rotate so the core's numerator shard (local rows b*512..(b+1)*512)
        # lands in tiles [0, 4) — the kernel always numerates its first 4.
        sblk = np.concatenate(
            [sblk[b * NSH : (b + 1) * NSH], sblk[: b * NSH], sblk[(b + 1) * NSH :]],
            axis=0,
        )
        bs = b8[c * NSH : (c + 1) * NSH]
        in_maps.append(
            {
                "sT_block": np.ascontiguousarray(sblk.T),
                "s_shard": np.ascontiguousarray(sblk[:NSH]),
                "r_block": np.ascontiguousarray(r8[b * RB : (b + 1) * RB]),
                "b_shard": np.ascontiguousarray(bs),
                "bT_shard": np.ascontiguousarray(bs.T),
            }
        )
    return in_maps


def _combine(results):
    denom = 0.0
    s_tot = np.zeros(D, dtype=np.float64)
    b_tot = np.zeros(D, dtype=np.float64)
    for r in results:
        denom += r["denom_acc"].astype(np.float64).sum()
        s_tot += r["ssum"][0].astype(np.float64)
        b_tot += r["bsum"][0].astype(np.float64)
    s_tot /= CSC  # numerator lhsT was fp8(C/||s||)
    b_tot /= CSC  # numerator lhsT was fp8(C/||b||)
    loss = np.log(denom) - (s_tot @ b_tot) / (float(N) * float(N)) / TEMP
    return np.array(loss, dtype=np.float32)


def _run(x_source, x_bc_target, x_raw_target, trace=False):
    nc = _get_nc()
    in_maps = _make_in_maps(x_source, x_bc_target, x_raw_target)
    res = run_bass_kernel_spmd(nc, in_maps, core_ids=list(range(8)), trace=trace)
    return _combine(res.results), res


def kernel(x_source, x_bc_target, x_raw_target):
    out, _ = _run(x_source, x_bc_target, x_raw_target)
    return out
